# revision 33
# baseline (speedup 1.0000x reference)
"""Causal self-attention (B=4, T=2048, H=8, hd=128, D=1024) on 8 trn2 cores.

Sharding: core c handles batch b = c//2, head-group g = c%2 (heads 4g..4g+4).
Each core computes qkv projection for its 4 heads, rms-norm + rope on q/k,
v = l0*v + l1*ve, causal attention, and a partial c_proj ([T, D]) over its
head group.  Host sums the two head-group partials per batch.

All matmuls run as float32r (TensorE full-rate fp32 mode).
"""
import sys

sys.path.insert(0, "/opt/trn_rl_repo")

import numpy as np

import concourse.bass as bass
import concourse.mybir as mybir
import concourse.tile as tile
from concourse import bacc
from concourse.bass import ts
from concourse.bass_utils import run_bass_kernel_spmd
from concourse.masks import make_identity

F32 = mybir.dt.float32
F32R = mybir.dt.float32r
MULT = mybir.AluOpType.mult
ADD = mybir.AluOpType.add
AF = mybir.ActivationFunctionType

# ---- problem constants (hardcoded per the contract) ----
B, T, D = 4, 2048, 1024
H, HD = 8, 128
HG = 4          # heads per group
EG = HG * HD    # 512 cols per head-group
ATTN_SCALE = 0.12
RMS_EPS = 1.1920929e-07
P = 128
NT = T // P     # 16 t-blocks
ND = D // P     # 8 d-chunks
NW = T // 512   # 4 query windows
S2 = ATTN_SCALE * ATTN_SCALE

_CACHED = {}


def _rope_tables():
    af = (1.0 / 1024.0) ** np.linspace(0.0, 1.0, HD // 4, dtype=np.float32)
    af = np.concatenate([af, np.zeros(HD // 4, dtype=np.float32)])
    t = np.arange(T, dtype=np.float32)
    theta = np.einsum("i,j->ij", t, af)  # [T, 64]
    cos, sin = np.cos(theta), np.sin(theta)
    # cc: [c|c] per head; ss: [s|-s] per head -> [T, 4, 128] -> [T, 512]
    cc1 = np.concatenate([cos, cos], axis=1)            # [T,128]
    ss1 = np.concatenate([sin, -sin], axis=1)           # [T,128]
    cc = np.tile(cc1, (1, HG)).astype(np.float32)       # [T,512]
    ss = np.tile(ss1, (1, HG)).astype(np.float32)       # [T,512]
    return cc, ss


def _masks():
    # tri[tk, c] = 1 if c >= tk  (the causal edge band)
    tk = np.arange(P)[:, None]
    c = np.arange(P)[None, :]
    return (c >= tk).astype(np.float32)


def build(cfg=None):
    cfg = cfg or {}
    ptp_bufs = cfg.get("ptp_bufs", 4)
    interleave_t = cfg.get("interleave_t", False)
    warmup = cfg.get("warmup", 16)
    delay_rs = cfg.get("delay_rs", 2)
    gp_bcast = cfg.get("gp_bcast", False)
    warmup = cfg.get("warmup", warmup)
    batch_a = cfg.get("batch_a", False)
    nc = bacc.Bacc("TRN2", target_bir_lowering=False, debug=False)

    xT = nc.dram_tensor("xT", [D, T], F32R, kind="ExternalInput")
    wqkT = nc.dram_tensor("wqkT", [D, 2 * EG], F32R, kind="ExternalInput")
    wvT = nc.dram_tensor("wvT", [D, EG], F32R, kind="ExternalInput")
    ve = nc.dram_tensor("ve", [T, EG], F32, kind="ExternalInput")
    cpT = nc.dram_tensor("cpT", [EG, D], F32R, kind="ExternalInput")
    cc_t = nc.dram_tensor("cc", [T, EG], F32, kind="ExternalInput")
    ss_t = nc.dram_tensor("ss", [T, EG], F32, kind="ExternalInput")
    mk_t = nc.dram_tensor("mk", [P, P], F32, kind="ExternalInput")
    sel_t = nc.dram_tensor("selm", [4, NW * P], F32R, kind="ExternalInput")
    out = nc.dram_tensor("out", [T, D], F32, kind="ExternalOutput")

    xTr = xT.rearrange("(c p) t -> c p t", p=P)       # [8, 128, 2048]
    wqkr = wqkT.rearrange("(c p) e -> c p e", p=P)    # [8, 128, 1024]
    wvr = wvT.rearrange("(c p) e -> c p e", p=P)      # [8, 128, 512]
    cpr = cpT.rearrange("(c p) d -> c p d", p=P)      # [4, 128, 1024]
    ver = ve.rearrange("(i p) e -> i p e", p=P)       # [16, 128, 512]
    ccr = cc_t.rearrange("(i p) e -> i p e", p=P)
    ssr = ss_t.rearrange("(i p) e -> i p e", p=P)

    with tile.TileContext(nc) as tc:
        with (
            tc.tile_pool(name="persist", bufs=1) as pp,
            tc.tile_pool(name="consts", bufs=1) as cp,
        ):
            # persistent tensors
            QT = [pp.tile([P, T], F32R, tag=f"QT{h}", name=f"QT{h}") for h in range(HG)]
            KT = [pp.tile([P, T], F32R, tag=f"KT{h}", name=f"KT{h}") for h in range(HG)]
            V = [pp.tile([P, EG], F32R, tag=f"V{i}", name=f"V{i}") for i in range(NT)]
            tri = cp.tile([P, P], F32, tag="tri")
            SK = cp.tile([P, NT, HG], F32, tag="SK")
            ident = cp.tile([P, P], F32, tag="ident")
            ones_col = cp.tile([P, 1], F32, tag="ones_col")
            ones_row = cp.tile([1, P], F32, tag="ones_row")
            bias_q = cp.tile([P, 1], F32, tag="bias_q")
            bias_k = cp.tile([P, 1], F32, tag="bias_k")
            nc.vector.memset(bias_q[:], RMS_EPS / S2)
            nc.vector.memset(bias_k[:], float(RMS_EPS))
            nc.sync.dma_start(tri[:], mk_t[:, :])
            nc.vector.memset(ones_col[:], 1.0)
            nc.vector.memset(ones_row[:], 1.0)
            make_identity(nc, ident[:])
            identr = cp.tile([P, P], F32R, tag="identr")
            nc.scalar.copy(identr[:], ident[:])

            # ---------------- Phase A: projections, rms+rope, transposes ---
            with (
                tc.tile_pool(name="wpool", bufs=1) as wp,
                tc.tile_pool(name="xpool", bufs=4) as xp,
                tc.tile_pool(name="qkte", bufs=2) as qp,
                tc.tile_pool(name="ropetmp", bufs=1) as rp,
                tc.tile_pool(name="rrpool", bufs=2) as rrp,
                tc.tile_pool(name="finpool", bufs=2) as fp,
                tc.tile_pool(name="tabs", bufs=2) as tp,
                tc.tile_pool(name="pA", bufs=2, space="PSUM") as pA,
                tc.tile_pool(name="pT", bufs=2, space="PSUM") as pT,
            ):
                xtis = {}

                def fetch_x(i):
                    if i < NT:
                        xi = xp.tile([P, ND, P], F32R, tag="xt", name="xt")
                        nc.sync.dma_start(
                            xi[:], xTr[:, :, ts(i, P)].rearrange("c p t -> p c t"))
                        xtis[i] = xi

                fetch_x(0)
                fetch_x(1)
                fetch_x(2)
                fetch_x(3)
                wqk = [wp.tile([P, 2 * EG], F32R, tag=f"wqk{c}", name=f"wqk{c}") for c in range(ND)]
                wv = [wp.tile([P, EG], F32R, tag=f"wv{c}", name=f"wv{c}") for c in range(ND)]
                for c in range(ND):
                    nc.sync.dma_start(wqk[c][:], wqkr[c])
                    nc.sync.dma_start(wv[c][:], wvr[c])

                if warmup:
                    wt = wp.tile([P, EG], F32, tag="warmsrc", name="warmsrc")
                    nc.vector.memset(wt[:], 0.0)
                    for wi in range(warmup):
                        pw = pA.tile([P, EG], F32, tag="psq", name="warm",
                                     bufs=3)
                        nc.tensor.matmul(pw[0:1, :], ones_col[:].bitcast(F32R),
                                         wt[:].bitcast(F32R), start=True, stop=True)

                pendA = None

                def emit_transposes(fin, rr_k, tsl):
                    for h in range(HG):
                        ptr = pT.tile([P, P], F32, tag="ptr", name="ptr")
                        nc.tensor.transpose(ptr[:].bitcast(F32R),
                                            fin[:, ts(h, HD)],
                                            identr[:])
                        nc.scalar.copy(QT[h][:, tsl], ptr[:])
                    for h in range(HG):
                        ptr = pT.tile([P, P], F32, tag="ptr", name="ptr")
                        nc.tensor.transpose(ptr[:], rr_k[:, ts(h, HD)],
                                            ident[:])
                        nc.vector.tensor_copy(KT[h][:, tsl], ptr[:])

                for i in range(NT):
                    tsl = ts(i, P)
                    xti = xtis.pop(i)

                    psq = pA.tile([P, EG], F32, tag="psq", bufs=3)
                    psk = pA.tile([P, EG], F32, tag="psk", bufs=2)
                    psv = pA.tile([P, EG], F32, tag="psv", bufs=1)
                    fetch_x(i + 4)
                    # q first: its (longer) elementwise chain starts while the
                    # k/v projections are still streaming on PE
                    for c in range(ND):
                        nc.tensor.matmul(psq[:], xti[:, c, :], wqk[c][:, 0:EG],
                                         start=(c == 0), stop=(c == ND - 1))
                    for c in range(ND):
                        nc.tensor.matmul(psk[:], xti[:, c, :], wqk[c][:, EG:2 * EG],
                                         start=(c == 0), stop=(c == ND - 1))
                    for c in range(ND):
                        nc.tensor.matmul(psv[:], xti[:, c, :], wv[c][:],
                                         start=(c == 0), stop=(c == ND - 1))

                    # --- v = psv + ve_scaled (lambdas folded on host) ---
                    vet = tp.tile([P, EG], F32, tag="vet")
                    nc.scalar.dma_start(vet[:], ver[i])
                    nc.vector.tensor_tensor(
                        V[i][:], psv[:], vet[:], op=ADD)

                    # --- k half to SBUF (gpsimd cannot read PSUM) ---
                    kte = qp.tile([P, EG], F32, tag="kte")
                    nc.scalar.copy(kte[:], psk[:])

                    # --- rms sumsq straight from PSUM ---
                    sq_scr = rp.tile([P, 2 * EG], F32, tag="sq_scr")
                    nc.scalar.activation(sq_scr[:, 0:EG], psq[:], AF.Square)
                    nc.scalar.activation(sq_scr[:, EG:2 * EG], psk[:], AF.Square)
                    ssq = rp.tile([P, 8], F32, tag="ssq")
                    nc.vector.tensor_reduce(
                        ssq[:], sq_scr[:].rearrange("p (g e) -> p g e", e=HD),
                        op=ADD, axis=mybir.AxisListType.X)
                    # scales: q gets 0.12 folded in; k scale is folded into the
                    # phase-B exp (per-partition scale), so only store recip.
                    sc = rp.tile([P, 8], F32, tag="sc")
                    nc.scalar.activation(sc[:, 0:4], ssq[:, 0:4], AF.Sqrt,
                                         scale=1.0 / (HD * S2), bias=bias_q[:])
                    nc.scalar.activation(sc[:, 4:8], ssq[:, 4:8], AF.Sqrt,
                                         scale=1.0 / HD, bias=bias_k[:])
                    rsc = rp.tile([P, 4], F32, tag="rsc")
                    nc.vector.reciprocal(rsc[:], sc[:, 0:4])
                    nc.vector.reciprocal(SK[:, i, :], sc[:, 4:8])

                    cct = tp.tile([P, EG], F32, tag="cct")
                    sst = tp.tile([P, EG], F32, tag="sst")
                    nc.gpsimd.dma_start(cct[:], ccr[i])
                    nc.sync.dma_start(sst[:], ssr[i])
                    c4 = cct[:].rearrange("p (h s e) -> p h s e", h=HG, s=2)
                    s4 = sst[:].rearrange("p (h s e) -> p h s e", h=HG, s=2)

                    def rope_side(eng, src_ap, tag):
                        x4 = src_ap.rearrange("p (h s e) -> p h s e", h=HG, s=2)
                        t1 = rp.tile([P, HG, 2, 64], F32, tag=f"t1_{tag}",
                                     name=f"t1_{tag}")
                        t2 = rp.tile([P, HG, 2, 64], F32, tag=f"t2_{tag}",
                                     name=f"t2_{tag}")
                        eng.tensor_tensor(t1[:], x4, c4, op=MULT)
                        eng.tensor_tensor(t2[:, :, 0, :], x4[:, :, 1, :],
                                          s4[:, :, 0, :], op=MULT)
                        eng.tensor_tensor(t2[:, :, 1, :], x4[:, :, 0, :],
                                          s4[:, :, 1, :], op=MULT)
                        rr = rrp.tile([P, EG], F32, tag=f"rr_{tag}",
                                      name=f"rr_{tag}")
                        r4 = rr[:].rearrange("p (h s e) -> p h s e", h=HG, s=2)
                        eng.tensor_tensor(r4, t1[:], t2[:], op=ADD)
                        return rr

                    # q: rope on DVE straight from PSUM, scale on ACT;
                    # k: rope on GPSIMD from the SBUF copy (no scale)
                    rr_q = rope_side(nc.vector, psq[:], "q")
                    fin = fp.tile([P, EG], F32R, tag="fin_q")
                    for h in range(HG):
                        nc.scalar.activation(fin[:, ts(h, HD)],
                                             rr_q[:, ts(h, HD)],
                                             AF.Copy, scale=rsc[:, h:h + 1])
                    rr_k = rope_side(nc.gpsimd, kte[:], "k")

                    # transposes of the PREVIOUS block go after this block's
                    # projections in the PE queue (hides the elementwise chain)
                    if pendA is not None:
                        emit_transposes(*pendA)
                    pendA = (fin, rr_k, tsl)
                emit_transposes(*pendA)

            # ---------------- Phase B: attention + c_proj (h-outer) ---------
            # Per head: key blocks j outer, all valid query windows batched.
            # exp runs on 1024/512-col PSUM chunks (amortizes the ~293ns ACT
            # fixed cost); KT_j / V_j stationary reused across windows.
            # Denominators accumulate in ONE [4,512] psum bank via one-hot
            # lhsT columns (row w = window w's colsums).
            with (
                tc.tile_pool(name="cpool", bufs=1) as cpl,
                tc.tile_pool(name="ypool", bufs=1) as yp,
            ):
                cpt = [cpl.tile([P, D], F32R, tag=f"cpt{e}", name=f"cpt{e}")
                       for e in range(HG)]
                for e in range(HG):
                    nc.sync.dma_start(cpt[e][:], cpr[e])
                em = cpl.tile([P, 7], F32, tag="em", name="em")
                nc.vector.memset(em[:], 0.0)
                nc.vector.memset(em[:, 3:4], 1.0)
                # row-selector lhsT tiles: sel[w] is [4,128] with row w ones
                selm = cpl.tile([4, NW * P], F32R, tag="selm", name="selm")
                nc.sync.dma_start(selm[:], sel_t[:, :])
                sel = [selm[:, ts(w, P)] for w in range(NW)]
                Yt = [yp.tile([P, T], F32R, tag=f"Y{h}", name=f"Y{h}")
                      for h in range(HG)]

                with (
                    tc.tile_pool(name="ptpool", bufs=5) as ptp,
                    tc.tile_pool(name="rpool", bufs=2) as rpl,
                    tc.tile_pool(name="pS", bufs=1, space="PSUM") as pS,
                    tc.tile_pool(name="pY", bufs=1, space="PSUM") as pY,
                    tc.tile_pool(name="pR", bufs=1, space="PSUM") as pR,
                ):
                    # Software pipeline: producers (scores+exp per (h,j)) run
                    # a few consumer-slots ahead of the PV consumers; the ones
                    # (denominator) matmuls have no downstream consumer until
                    # head-end, so they sit in a deferred queue popped as PE
                    # filler work.  Keeps PE dense so HAM never re-throttles.
                    LAG = 2
                    ONES_RESERVE = 4
                    cons_q = []
                    ones_q = []

                    class _St:
                        pass

                    def ensure_acc(st):
                        if st.ps_y is None:
                            st.ps_y = [pY.tile([P, 512], F32, tag=f"psy{w}",
                                               name=f"psy{w}")
                                       for w in range(NW)]
                            st.ps_r = pR.tile([4, 512], F32, tag="psr",
                                              name="psr")

                    def consume(st, j, pt):
                        ensure_acc(st)
                        h = st.h
                        for w in range(j // 4, NW):
                            lo = max(512 * w, P * j)
                            loc, po = lo - P * j, lo - 512 * w
                            width = 512 * (w + 1) - lo
                            nc.tensor.matmul(
                                st.ps_y[w][:, po:512], V[j][:, ts(h, HD)],
                                pt[:, loc:loc + width],
                                start=(j == 0), stop=(j == 4 * w + 3))

                    def one_piece(st, j, w, pt):
                        ensure_acc(st)
                        lo = max(512 * w, P * j)
                        loc, po = lo - P * j, lo - 512 * w
                        width = 512 * (w + 1) - lo
                        nc.tensor.matmul(
                            st.ps_r[:, po:512],
                            em[:, 3 - w:7 - w].bitcast(F32R),
                            pt[:, loc:loc + width],
                            start=st.rstart,
                            stop=(j == NT - 1 and w == NW - 1))
                        st.rstart = False

                    def norm(st):
                        h = st.h
                        rro = rpl.tile([4, 512], F32, tag="rro", name="rro")
                        nc.vector.reciprocal_approx_fast(rro[:], st.ps_r[:])
                        rrow = rpl.tile([4, 512], F32R, tag="rrow", name="rrow")
                        nc.vector.tensor_copy(rrow[:], rro[:])
                        for w in range(NW):
                            tagb = "psA" if (w % 2 == 0) else "psB"
                            shp = 1024 if w % 2 == 0 else 512
                            ps_b = pS.tile([P, shp], F32, tag=tagb, name="ps_b")
                            nc.tensor.matmul(ps_b[:, 0:512], sel[w], rrow[:],
                                             start=True, stop=True)
                            bb = rpl.tile([P, 512], F32, tag="bb", name="bb")
                            nc.vector.tensor_copy(bb[:], ps_b[:, 0:512])
                            nc.vector.tensor_tensor(Yt[h][:, ts(w, 512)],
                                                    st.ps_y[w][:], bb[:],
                                                    op=MULT)

                    for h in range(HG):
                        st = _St()
                        st.h, st.ps_y, st.ps_r, st.rstart = h, None, None, True
                        for j in range(NT):
                            cols = T - P * j
                            pt = ptp.tile([P, T], F32R, tag="pt", name="pt")
                            # alternate the leading PSUM buffer per j so both
                            # chunk buffers stay in rotation (pipeline depth 2)
                            cap_a = 1024 if cols > 1024 else 512
                            seq = ("psA", "psB") if j % 2 == 0 else \
                                  ("psB", "psA")
                            chunks = []
                            off = 0
                            k = 0
                            while off < cols:
                                tag = seq[k % 2]
                                k += 1
                                size = min(cap_a if tag == "psA" else 512,
                                           cols - off)
                                chunks.append((off, size, tag,
                                               1024 if tag == "psA" else 512))
                                off += size

                            def sc_chunk(off, csz, tag, shp, j=j, pt=pt, h=h):
                                ps = pS.tile([P, shp], F32, tag=tag, name=tag)
                                for s0 in range(0, csz, 512):
                                    sw = min(512, csz - s0)
                                    nc.tensor.matmul(
                                        ps[:, s0:s0 + sw], KT[h][:, ts(j, P)],
                                        QT[h][:, P * j + off + s0:
                                               P * j + off + s0 + sw],
                                        start=True, stop=True)
                                nc.scalar.activation(
                                    pt[:, off:off + csz], ps[:, 0:csz], AF.Exp,
                                    scale=SK[:, j, h:h + 1])

                            sc_chunk(*chunks[0])
                            # causal mask on the diagonal 128-col band
                            nc.gpsimd.tensor_tensor(pt[:, 0:P], pt[:, 0:P],
                                                    tri[:], op=MULT)
                            if len(ones_q) > ONES_RESERVE:
                                ones_q.pop(0)()
                            if len(chunks) > 1:
                                sc_chunk(*chunks[1])
                            while len(cons_q) > LAG:
                                cons_q.pop(0)()
                            if len(ones_q) > ONES_RESERVE:
                                ones_q.pop(0)()
                            for ch in chunks[2:]:
                                sc_chunk(*ch)
                            cons_q.append(
                                lambda st=st, j=j, pt=pt: consume(st, j, pt))
                            for w in range(j // 4, NW):
                                ones_q.append(
                                    lambda st=st, j=j, w=w, pt=pt:
                                    one_piece(st, j, w, pt))

                        def flush_ones(st=st):
                            while ones_q:
                                ones_q.pop(0)()
                        cons_q.append(flush_ones)
                        cons_q.append(lambda st=st: norm(st))
                    while cons_q:
                        cons_q.pop(0)()

                # ---- c_proj tail: out[tq, :] = sum_h Y_h^T @ cpt_h ----
                with (
                    tc.tile_pool(name="opool", bufs=3) as op_,
                    tc.tile_pool(name="pO", bufs=2, space="PSUM") as pO,
                ):
                    for w in range(NW):
                        for tb in range(4):
                            base = w * 512 + tb * P
                            po0 = pO.tile([P, 512], F32, tag="po0", name="po0")
                            po1 = pO.tile([P, 512], F32, tag="po1", name="po1")
                            for h in range(HG):
                                nc.tensor.matmul(po0[:],
                                                 Yt[h][:, base:base + P],
                                                 cpt[h][:, 0:512],
                                                 start=(h == 0),
                                                 stop=(h == HG - 1))
                                nc.tensor.matmul(po1[:],
                                                 Yt[h][:, base:base + P],
                                                 cpt[h][:, 512:1024],
                                                 start=(h == 0),
                                                 stop=(h == HG - 1))
                            oc0 = op_.tile([P, 512], F32, tag="oc0", name="oc0")
                            oc1 = op_.tile([P, 512], F32, tag="oc1", name="oc1")
                            nc.vector.tensor_copy(oc0[:], po0[:])
                            nc.sync.dma_start(out[base:base + P, 0:512], oc0[:])
                            nc.scalar.copy(oc1[:], po1[:])
                            nc.sync.dma_start(out[base:base + P, 512:1024],
                                              oc1[:])
    nc.compile()
    return nc


def _get_nc():
    if "nc" not in _CACHED:
        _CACHED["nc"] = build()
    return _CACHED["nc"]


def _try_install_profile_shim():
    try:
        import contextlib
        import ctypes
        import types

        if "antenv.axon_hooks" in sys.modules:
            return
        so_path = "/opt/axon/libaxon_pjrt.so"
        lib = ctypes.CDLL(so_path)
        if not hasattr(lib, "axon_start_nrt_profile"):
            return
        lib.axon_start_nrt_profile.argtypes = [ctypes.POINTER(ctypes.c_int64),
                                               ctypes.c_size_t]
        lib.axon_start_nrt_profile.restype = ctypes.c_int64
        lib.axon_stop_nrt_profile.argtypes = [ctypes.c_char_p]
        lib.axon_stop_nrt_profile.restype = ctypes.c_int64

        @contextlib.contextmanager
        def _hook(output_dir, device_ids):
            import jax

            jax.devices()
            if device_ids:
                ids = (ctypes.c_int64 * len(device_ids))(*device_ids)
                rc = lib.axon_start_nrt_profile(ids, len(device_ids))
            else:
                rc = lib.axon_start_nrt_profile(None, 0)
            if rc != 0:
                raise RuntimeError(f"axon_start_nrt_profile rc={rc}")
            try:
                yield
            finally:
                lib.axon_stop_nrt_profile(str(output_dir).encode())

        mod = types.ModuleType("antenv.axon_hooks")
        mod.set_axon_ntff_profile_hook = lambda h: None
        mod.get_axon_ntff_profile_hook = lambda: _hook
        import antenv

        antenv.axon_hooks = mod
        sys.modules["antenv.axon_hooks"] = mod
    except Exception:
        pass


LAST_EXEC_TIME_NS = None


def _prepare_in_maps(x, ve, sa_lambdas, qkv_w, c_proj_weight):
    x = np.asarray(x, dtype=np.float32)
    ve = np.asarray(ve, dtype=np.float32)
    sa_lambdas = np.asarray(sa_lambdas, dtype=np.float32)
    qkv_w = np.asarray(qkv_w, dtype=np.float32)
    c_proj_weight = np.asarray(c_proj_weight, dtype=np.float32)

    cc, ss = _rope_tables()
    mk = _masks()
    l0, l1 = float(sa_lambdas[0]), float(sa_lambdas[1])
    selm = np.zeros((4, 4 * P), dtype=np.float32)
    for w in range(4):
        selm[w, w * P:(w + 1) * P] = 1.0

    in_maps = []
    for c in range(8):
        b, g = c // 2, c % 2
        gs, ge = g * EG, (g + 1) * EG
        wq = qkv_w[0, gs:ge, :]           # [512, 1024]
        wk = qkv_w[1, gs:ge, :]
        wv = qkv_w[2, gs:ge, :] * l0      # fold lambda0 into the v projection
        in_maps.append({
            "xT": np.ascontiguousarray(x[b].T),                       # [D, T]
            "wqkT": np.ascontiguousarray(
                np.concatenate([wq, wk], axis=0).T),                  # [D, 1024]
            "wvT": np.ascontiguousarray(wv.T),                        # [D, 512]
            "ve": np.ascontiguousarray(
                ve[b].reshape(T, H, HD)[:, g * HG:(g + 1) * HG, :]
                .reshape(T, EG) * l1),                                # [T, 512]
            "cpT": np.ascontiguousarray(c_proj_weight[:, gs:ge].T),   # [512, D]
            "cc": cc, "ss": ss, "mk": mk, "selm": selm,
        })
    return in_maps


def kernel(x, ve, sa_lambdas, qkv_w, c_proj_weight):
    global LAST_EXEC_TIME_NS
    in_maps = _prepare_in_maps(x, ve, sa_lambdas, qkv_w, c_proj_weight)
    _try_install_profile_shim()
    nc = _get_nc()
    res = run_bass_kernel_spmd(nc, in_maps, core_ids=list(range(8)), trace=True)
    LAST_EXEC_TIME_NS = res.exec_time_ns

    outs = [res.results[c]["out"] for c in range(8)]
    full = np.stack([outs[2 * b] + outs[2 * b + 1] for b in range(B)], axis=0)
    return full.astype(np.float32)



# revision 34
# speedup vs baseline: 1.0161x; 1.0161x over previous
"""Causal self-attention (B=4, T=2048, H=8, hd=128, D=1024) on 8 trn2 cores.

Sharding: core c handles batch b = c//2, head-group g = c%2 (heads 4g..4g+4).
Each core computes qkv projection for its 4 heads, rms-norm + rope on q/k,
v = l0*v + l1*ve, causal attention, and a partial c_proj ([T, D]) over its
head group.  Host sums the two head-group partials per batch.

All matmuls run as float32r (TensorE full-rate fp32 mode).
"""
import sys

sys.path.insert(0, "/opt/trn_rl_repo")

import numpy as np

import concourse.bass as bass
import concourse.mybir as mybir
import concourse.tile as tile
from concourse import bacc
from concourse.bass import ts
from concourse.bass_utils import run_bass_kernel_spmd
from concourse.masks import make_identity

F32 = mybir.dt.float32
F32R = mybir.dt.float32r
MULT = mybir.AluOpType.mult
ADD = mybir.AluOpType.add
AF = mybir.ActivationFunctionType

# ---- problem constants (hardcoded per the contract) ----
B, T, D = 4, 2048, 1024
H, HD = 8, 128
HG = 4          # heads per group
EG = HG * HD    # 512 cols per head-group
ATTN_SCALE = 0.12
RMS_EPS = 1.1920929e-07
P = 128
NT = T // P     # 16 t-blocks
ND = D // P     # 8 d-chunks
NW = T // 512   # 4 query windows
S2 = ATTN_SCALE * ATTN_SCALE

_CACHED = {}


def _rope_tables():
    af = (1.0 / 1024.0) ** np.linspace(0.0, 1.0, HD // 4, dtype=np.float32)
    af = np.concatenate([af, np.zeros(HD // 4, dtype=np.float32)])
    t = np.arange(T, dtype=np.float32)
    theta = np.einsum("i,j->ij", t, af)  # [T, 64]
    cos, sin = np.cos(theta), np.sin(theta)
    # cc: [c|c] per head; ss: [s|-s] per head -> [T, 4, 128] -> [T, 512]
    cc1 = np.concatenate([cos, cos], axis=1)            # [T,128]
    ss1 = np.concatenate([sin, -sin], axis=1)           # [T,128]
    cc = np.tile(cc1, (1, HG)).astype(np.float32)       # [T,512]
    ss = np.tile(ss1, (1, HG)).astype(np.float32)       # [T,512]
    return cc, ss


def _masks():
    # tri[tk, c] = 1 if c >= tk  (the causal edge band)
    tk = np.arange(P)[:, None]
    c = np.arange(P)[None, :]
    return (c >= tk).astype(np.float32)


def build(cfg=None):
    cfg = cfg or {}
    ptp_bufs = cfg.get("ptp_bufs", 4)
    interleave_t = cfg.get("interleave_t", False)
    warmup = cfg.get("warmup", 16)
    delay_rs = cfg.get("delay_rs", 2)
    gp_bcast = cfg.get("gp_bcast", False)
    warmup = cfg.get("warmup", warmup)
    batch_a = cfg.get("batch_a", False)
    nc = bacc.Bacc("TRN2", target_bir_lowering=False, debug=False)

    xT = nc.dram_tensor("xT", [D, T], F32R, kind="ExternalInput")
    wqkT = nc.dram_tensor("wqkT", [D, 2 * EG], F32R, kind="ExternalInput")
    wvT = nc.dram_tensor("wvT", [D, EG], F32R, kind="ExternalInput")
    ve = nc.dram_tensor("ve", [T, EG], F32, kind="ExternalInput")
    cpT = nc.dram_tensor("cpT", [EG, D], F32R, kind="ExternalInput")
    cc_t = nc.dram_tensor("cc", [T, EG], F32, kind="ExternalInput")
    ss_t = nc.dram_tensor("ss", [T, EG], F32, kind="ExternalInput")
    mk_t = nc.dram_tensor("mk", [P, P], F32, kind="ExternalInput")
    sel_t = nc.dram_tensor("selm", [4, NW * P], F32R, kind="ExternalInput")
    out = nc.dram_tensor("out", [T, D], F32, kind="ExternalOutput")

    xTr = xT.rearrange("(c p) t -> c p t", p=P)       # [8, 128, 2048]
    wqkr = wqkT.rearrange("(c p) e -> c p e", p=P)    # [8, 128, 1024]
    wvr = wvT.rearrange("(c p) e -> c p e", p=P)      # [8, 128, 512]
    cpr = cpT.rearrange("(c p) d -> c p d", p=P)      # [4, 128, 1024]
    ver = ve.rearrange("(i p) e -> i p e", p=P)       # [16, 128, 512]
    ccr = cc_t.rearrange("(i p) e -> i p e", p=P)
    ssr = ss_t.rearrange("(i p) e -> i p e", p=P)

    with tile.TileContext(nc) as tc:
        with (
            tc.tile_pool(name="persist", bufs=1) as pp,
            tc.tile_pool(name="consts", bufs=1) as cp,
        ):
            # persistent tensors
            QT = [pp.tile([P, T], F32R, tag=f"QT{h}", name=f"QT{h}") for h in range(HG)]
            KT = [pp.tile([P, T], F32R, tag=f"KT{h}", name=f"KT{h}") for h in range(HG)]
            V = [pp.tile([P, EG], F32R, tag=f"V{i}", name=f"V{i}") for i in range(NT)]
            tri = cp.tile([P, P], F32, tag="tri")
            SK = cp.tile([P, NT, HG], F32, tag="SK")
            ident = cp.tile([P, P], F32, tag="ident")
            ones_col = cp.tile([P, 1], F32, tag="ones_col")
            ones_row = cp.tile([1, P], F32, tag="ones_row")
            bias_q = cp.tile([P, 1], F32, tag="bias_q")
            bias_k = cp.tile([P, 1], F32, tag="bias_k")
            nc.vector.memset(bias_q[:], RMS_EPS / S2)
            nc.vector.memset(bias_k[:], float(RMS_EPS))
            nc.sync.dma_start(tri[:], mk_t[:, :])
            nc.vector.memset(ones_col[:], 1.0)
            nc.vector.memset(ones_row[:], 1.0)
            make_identity(nc, ident[:])
            identr = cp.tile([P, P], F32R, tag="identr")
            nc.scalar.copy(identr[:], ident[:])

            # ---------------- Phase A: projections, rms+rope, transposes ---
            with (
                tc.tile_pool(name="wpool", bufs=1) as wp,
                tc.tile_pool(name="xpool", bufs=4) as xp,
                tc.tile_pool(name="qkte", bufs=2) as qp,
                tc.tile_pool(name="ropetmp", bufs=1) as rp,
                tc.tile_pool(name="rrpool", bufs=2) as rrp,
                tc.tile_pool(name="finpool", bufs=2) as fp,
                tc.tile_pool(name="tabs", bufs=2) as tp,
                tc.tile_pool(name="pA", bufs=2, space="PSUM") as pA,
                tc.tile_pool(name="pT", bufs=2, space="PSUM") as pT,
            ):
                xtis = {}

                def fetch_x(i):
                    if i < NT:
                        xi = xp.tile([P, ND, P], F32R, tag="xt", name="xt")
                        nc.sync.dma_start(
                            xi[:], xTr[:, :, ts(i, P)].rearrange("c p t -> p c t"))
                        xtis[i] = xi

                fetch_x(0)
                fetch_x(1)
                fetch_x(2)
                fetch_x(3)
                wqk = [wp.tile([P, 2 * EG], F32R, tag=f"wqk{c}", name=f"wqk{c}") for c in range(ND)]
                wv = [wp.tile([P, EG], F32R, tag=f"wv{c}", name=f"wv{c}") for c in range(ND)]
                for c in range(ND):
                    nc.sync.dma_start(wqk[c][:], wqkr[c])
                    nc.sync.dma_start(wv[c][:], wvr[c])

                if warmup:
                    wt = wp.tile([P, EG], F32, tag="warmsrc", name="warmsrc")
                    nc.vector.memset(wt[:], 0.0)
                    for wi in range(warmup):
                        pw = pA.tile([P, EG], F32, tag="psq", name="warm",
                                     bufs=3)
                        nc.tensor.matmul(pw[0:1, :], ones_col[:].bitcast(F32R),
                                         wt[:].bitcast(F32R), start=True, stop=True)

                pendA = None

                def emit_transposes(fin, rr_k, tsl):
                    for h in range(HG):
                        ptr = pT.tile([P, P], F32, tag="ptr", name="ptr")
                        nc.tensor.transpose(ptr[:].bitcast(F32R),
                                            fin[:, ts(h, HD)],
                                            identr[:])
                        nc.scalar.copy(QT[h][:, tsl], ptr[:])
                    for h in range(HG):
                        ptr = pT.tile([P, P], F32, tag="ptr", name="ptr")
                        nc.tensor.transpose(ptr[:], rr_k[:, ts(h, HD)],
                                            ident[:])
                        nc.vector.tensor_copy(KT[h][:, tsl], ptr[:])

                for i in range(NT):
                    tsl = ts(i, P)
                    xti = xtis.pop(i)

                    psq = pA.tile([P, EG], F32, tag="psq", bufs=3)
                    psk = pA.tile([P, EG], F32, tag="psk", bufs=2)
                    psv = pA.tile([P, EG], F32, tag="psv", bufs=1)
                    fetch_x(i + 4)
                    # q first: its (longer) elementwise chain starts while the
                    # k/v projections are still streaming on PE
                    for c in range(ND):
                        nc.tensor.matmul(psq[:], xti[:, c, :], wqk[c][:, 0:EG],
                                         start=(c == 0), stop=(c == ND - 1))
                    for c in range(ND):
                        nc.tensor.matmul(psk[:], xti[:, c, :], wqk[c][:, EG:2 * EG],
                                         start=(c == 0), stop=(c == ND - 1))
                    for c in range(ND):
                        nc.tensor.matmul(psv[:], xti[:, c, :], wv[c][:],
                                         start=(c == 0), stop=(c == ND - 1))

                    # --- v = psv + ve_scaled (lambdas folded on host) ---
                    vet = tp.tile([P, EG], F32, tag="vet")
                    nc.scalar.dma_start(vet[:], ver[i])
                    nc.vector.tensor_tensor(
                        V[i][:], psv[:], vet[:], op=ADD)

                    # --- k half to SBUF (gpsimd cannot read PSUM) ---
                    kte = qp.tile([P, EG], F32, tag="kte")
                    nc.scalar.copy(kte[:], psk[:])

                    # --- rms sumsq straight from PSUM ---
                    sq_scr = rp.tile([P, 2 * EG], F32, tag="sq_scr")
                    nc.scalar.activation(sq_scr[:, 0:EG], psq[:], AF.Square)
                    nc.scalar.activation(sq_scr[:, EG:2 * EG], psk[:], AF.Square)
                    ssq = rp.tile([P, 8], F32, tag="ssq")
                    nc.vector.tensor_reduce(
                        ssq[:], sq_scr[:].rearrange("p (g e) -> p g e", e=HD),
                        op=ADD, axis=mybir.AxisListType.X)
                    # scales: q gets 0.12 folded in; k scale is folded into the
                    # phase-B exp (per-partition scale), so only store recip.
                    sc = rp.tile([P, 8], F32, tag="sc")
                    nc.scalar.activation(sc[:, 0:4], ssq[:, 0:4], AF.Sqrt,
                                         scale=1.0 / (HD * S2), bias=bias_q[:])
                    nc.scalar.activation(sc[:, 4:8], ssq[:, 4:8], AF.Sqrt,
                                         scale=1.0 / HD, bias=bias_k[:])
                    rsc = rp.tile([P, 4], F32, tag="rsc")
                    nc.vector.reciprocal(rsc[:], sc[:, 0:4])
                    nc.vector.reciprocal(SK[:, i, :], sc[:, 4:8])

                    cct = tp.tile([P, EG], F32, tag="cct")
                    sst = tp.tile([P, EG], F32, tag="sst")
                    nc.gpsimd.dma_start(cct[:], ccr[i])
                    nc.sync.dma_start(sst[:], ssr[i])
                    c4 = cct[:].rearrange("p (h s e) -> p h s e", h=HG, s=2)
                    s4 = sst[:].rearrange("p (h s e) -> p h s e", h=HG, s=2)

                    def rope_side(eng, src_ap, tag):
                        x4 = src_ap.rearrange("p (h s e) -> p h s e", h=HG, s=2)
                        t1 = rp.tile([P, HG, 2, 64], F32, tag=f"t1_{tag}",
                                     name=f"t1_{tag}")
                        t2 = rp.tile([P, HG, 2, 64], F32, tag=f"t2_{tag}",
                                     name=f"t2_{tag}")
                        eng.tensor_tensor(t1[:], x4, c4, op=MULT)
                        eng.tensor_tensor(t2[:, :, 0, :], x4[:, :, 1, :],
                                          s4[:, :, 0, :], op=MULT)
                        eng.tensor_tensor(t2[:, :, 1, :], x4[:, :, 0, :],
                                          s4[:, :, 1, :], op=MULT)
                        rr = rrp.tile([P, EG], F32, tag=f"rr_{tag}",
                                      name=f"rr_{tag}")
                        r4 = rr[:].rearrange("p (h s e) -> p h s e", h=HG, s=2)
                        eng.tensor_tensor(r4, t1[:], t2[:], op=ADD)
                        return rr

                    # q: rope on DVE straight from PSUM, scale on ACT;
                    # k: rope on GPSIMD from the SBUF copy (no scale)
                    rr_q = rope_side(nc.vector, psq[:], "q")
                    fin = fp.tile([P, EG], F32R, tag="fin_q")
                    for h in range(HG):
                        nc.scalar.activation(fin[:, ts(h, HD)],
                                             rr_q[:, ts(h, HD)],
                                             AF.Copy, scale=rsc[:, h:h + 1])
                    rr_k = rope_side(nc.gpsimd, kte[:], "k")

                    # transposes of the PREVIOUS block go after this block's
                    # projections in the PE queue (hides the elementwise chain)
                    if pendA is not None:
                        emit_transposes(*pendA)
                    pendA = (fin, rr_k, tsl)
                emit_transposes(*pendA)

            # ---------------- Phase B: attention + c_proj (h-outer) ---------
            # Per head: key blocks j outer, all valid query windows batched.
            # exp runs on 1024/512-col PSUM chunks (amortizes the ~293ns ACT
            # fixed cost); KT_j / V_j stationary reused across windows.
            # Denominators accumulate in ONE [4,512] psum bank via one-hot
            # lhsT columns (row w = window w's colsums).
            with (
                tc.tile_pool(name="cpool", bufs=1) as cpl,
                tc.tile_pool(name="ypool", bufs=1) as yp,
            ):
                cpt = [cpl.tile([P, D], F32R, tag=f"cpt{e}", name=f"cpt{e}")
                       for e in range(HG)]
                for e in range(HG):
                    nc.sync.dma_start(cpt[e][:], cpr[e])
                em = cpl.tile([P, 7], F32, tag="em", name="em")
                nc.vector.memset(em[:], 0.0)
                nc.vector.memset(em[:, 3:4], 1.0)
                # row-selector lhsT tiles: sel[w] is [4,128] with row w ones
                selm = cpl.tile([4, NW * P], F32R, tag="selm", name="selm")
                nc.sync.dma_start(selm[:], sel_t[:, :])
                sel = [selm[:, ts(w, P)] for w in range(NW)]
                Yt = [yp.tile([P, T], F32R, tag=f"Y{h}", name=f"Y{h}")
                      for h in range(HG)]

                with (
                    tc.tile_pool(name="ptpool", bufs=5) as ptp,
                    tc.tile_pool(name="rpool", bufs=2) as rpl,
                    tc.tile_pool(name="pS", bufs=1, space="PSUM") as pS,
                    tc.tile_pool(name="pY", bufs=1, space="PSUM") as pY,
                    tc.tile_pool(name="pR", bufs=1, space="PSUM") as pR,
                ):
                    # Software pipeline: producers (scores+exp per (h,j)) run
                    # a few consumer-slots ahead of the PV consumers; the ones
                    # (denominator) matmuls have no downstream consumer until
                    # head-end, so they sit in a deferred queue popped as PE
                    # filler work.  Keeps PE dense so HAM never re-throttles.
                    LAG = 2
                    ONES_RESERVE = 4
                    cons_q = []
                    ones_q = []

                    class _St:
                        pass

                    def ensure_acc(st):
                        if st.ps_y is None:
                            st.ps_y = [pY.tile([P, 512], F32, tag=f"psy{w}",
                                               name=f"psy{w}")
                                       for w in range(NW)]
                            st.ps_r = pR.tile([4, 512], F32, tag="psr",
                                              name="psr")

                    def consume(st, j, pt):
                        ensure_acc(st)
                        h = st.h
                        for w in range(j // 4, NW):
                            lo = max(512 * w, P * j)
                            loc, po = lo - P * j, lo - 512 * w
                            width = 512 * (w + 1) - lo
                            nc.tensor.matmul(
                                st.ps_y[w][:, po:512], V[j][:, ts(h, HD)],
                                pt[:, loc:loc + width],
                                start=(j == 0), stop=(j == 4 * w + 3))

                    def one_piece(st, j, w, pt):
                        ensure_acc(st)
                        lo = max(512 * w, P * j)
                        loc, po = lo - P * j, lo - 512 * w
                        width = 512 * (w + 1) - lo
                        nc.tensor.matmul(
                            st.ps_r[:, po:512],
                            em[:, 3 - w:7 - w].bitcast(F32R),
                            pt[:, loc:loc + width],
                            start=st.rstart,
                            stop=(j == NT - 1 and w == NW - 1))
                        st.rstart = False

                    def norm(st):
                        h = st.h
                        rro = rpl.tile([4, 512], F32, tag="rro", name="rro")
                        nc.vector.reciprocal_approx_fast(rro[:], st.ps_r[:])
                        rrow = rpl.tile([4, 512], F32R, tag="rrow", name="rrow")
                        nc.vector.tensor_copy(rrow[:], rro[:])
                        for w in range(NW):
                            tagb = "psA" if (w % 2 == 0) else "psB"
                            shp = 1024 if w % 2 == 0 else 512
                            ps_b = pS.tile([P, shp], F32, tag=tagb, name="ps_b")
                            nc.tensor.matmul(ps_b[:, 0:512], sel[w], rrow[:],
                                             start=True, stop=True)
                            bb = rpl.tile([P, 512], F32, tag="bb", name="bb")
                            nc.vector.tensor_copy(bb[:], ps_b[:, 0:512])
                            nc.vector.tensor_tensor(Yt[h][:, ts(w, 512)],
                                                    st.ps_y[w][:], bb[:],
                                                    op=MULT)

                    for h in range(HG):
                        st = _St()
                        st.h, st.ps_y, st.ps_r, st.rstart = h, None, None, True
                        for j in range(NT):
                            cols = T - P * j
                            pt = ptp.tile([P, T], F32R, tag="pt", name="pt")
                            # alternate the leading PSUM buffer per j so both
                            # chunk buffers stay in rotation (pipeline depth 2)
                            if cols > 1024:
                                seq = ("psA", "psB") if j % 2 == 0 else \
                                      ("psB", "psA")
                                chunks = []
                                off = 0
                                k = 0
                                while off < cols:
                                    tag = seq[k % 2]
                                    k += 1
                                    size = min(1024 if tag == "psA" else 512,
                                               cols - off)
                                    chunks.append(
                                        (off, size, tag,
                                         1024 if tag == "psA" else 512))
                                    off += size
                            elif cols > 512:
                                chunks = [(0, cols, "psA", 1024)]
                            else:
                                tag = "psA" if j % 2 == 0 else "psB"
                                chunks = [(0, cols, tag,
                                           1024 if tag == "psA" else 512)]

                            def sc_chunk(off, csz, tag, shp, j=j, pt=pt, h=h):
                                ps = pS.tile([P, shp], F32, tag=tag, name=tag)
                                for s0 in range(0, csz, 512):
                                    sw = min(512, csz - s0)
                                    nc.tensor.matmul(
                                        ps[:, s0:s0 + sw], KT[h][:, ts(j, P)],
                                        QT[h][:, P * j + off + s0:
                                               P * j + off + s0 + sw],
                                        start=True, stop=True)
                                nc.scalar.activation(
                                    pt[:, off:off + csz], ps[:, 0:csz], AF.Exp,
                                    scale=SK[:, j, h:h + 1])

                            # guaranteed-ready filler right before the
                            # (dependency-waiting) first chunk matmul
                            if len(ones_q) > ONES_RESERVE:
                                ones_q.pop(0)()
                            sc_chunk(*chunks[0])
                            # causal mask on the diagonal 128-col band
                            nc.gpsimd.tensor_tensor(pt[:, 0:P], pt[:, 0:P],
                                                    tri[:], op=MULT)
                            if len(chunks) > 1:
                                sc_chunk(*chunks[1])
                            while len(cons_q) > LAG:
                                cons_q.pop(0)()
                            if len(ones_q) > ONES_RESERVE:
                                ones_q.pop(0)()
                            for ch in chunks[2:]:
                                sc_chunk(*ch)
                            cons_q.append(
                                lambda st=st, j=j, pt=pt: consume(st, j, pt))
                            for w in range(j // 4, NW):
                                ones_q.append(
                                    lambda st=st, j=j, w=w, pt=pt:
                                    one_piece(st, j, w, pt))

                        def flush_ones(st=st):
                            while ones_q:
                                ones_q.pop(0)()
                        cons_q.append(flush_ones)
                        cons_q.append(lambda st=st: norm(st))
                    while cons_q:
                        cons_q.pop(0)()

                # ---- c_proj tail: out[tq, :] = sum_h Y_h^T @ cpt_h ----
                with (
                    tc.tile_pool(name="opool", bufs=3) as op_,
                    tc.tile_pool(name="pO", bufs=2, space="PSUM") as pO,
                ):
                    for w in range(NW):
                        for tb in range(4):
                            base = w * 512 + tb * P
                            po0 = pO.tile([P, 512], F32, tag="po0", name="po0")
                            po1 = pO.tile([P, 512], F32, tag="po1", name="po1")
                            for h in range(HG):
                                nc.tensor.matmul(po0[:],
                                                 Yt[h][:, base:base + P],
                                                 cpt[h][:, 0:512],
                                                 start=(h == 0),
                                                 stop=(h == HG - 1))
                                nc.tensor.matmul(po1[:],
                                                 Yt[h][:, base:base + P],
                                                 cpt[h][:, 512:1024],
                                                 start=(h == 0),
                                                 stop=(h == HG - 1))
                            oc0 = op_.tile([P, 512], F32, tag="oc0", name="oc0")
                            oc1 = op_.tile([P, 512], F32, tag="oc1", name="oc1")
                            nc.vector.tensor_copy(oc0[:], po0[:])
                            nc.sync.dma_start(out[base:base + P, 0:512], oc0[:])
                            nc.scalar.copy(oc1[:], po1[:])
                            nc.sync.dma_start(out[base:base + P, 512:1024],
                                              oc1[:])
    nc.compile()
    return nc


def _get_nc():
    if "nc" not in _CACHED:
        _CACHED["nc"] = build()
    return _CACHED["nc"]


def _try_install_profile_shim():
    try:
        import contextlib
        import ctypes
        import types

        if "antenv.axon_hooks" in sys.modules:
            return
        so_path = "/opt/axon/libaxon_pjrt.so"
        lib = ctypes.CDLL(so_path)
        if not hasattr(lib, "axon_start_nrt_profile"):
            return
        lib.axon_start_nrt_profile.argtypes = [ctypes.POINTER(ctypes.c_int64),
                                               ctypes.c_size_t]
        lib.axon_start_nrt_profile.restype = ctypes.c_int64
        lib.axon_stop_nrt_profile.argtypes = [ctypes.c_char_p]
        lib.axon_stop_nrt_profile.restype = ctypes.c_int64

        @contextlib.contextmanager
        def _hook(output_dir, device_ids):
            import jax

            jax.devices()
            if device_ids:
                ids = (ctypes.c_int64 * len(device_ids))(*device_ids)
                rc = lib.axon_start_nrt_profile(ids, len(device_ids))
            else:
                rc = lib.axon_start_nrt_profile(None, 0)
            if rc != 0:
                raise RuntimeError(f"axon_start_nrt_profile rc={rc}")
            try:
                yield
            finally:
                lib.axon_stop_nrt_profile(str(output_dir).encode())

        mod = types.ModuleType("antenv.axon_hooks")
        mod.set_axon_ntff_profile_hook = lambda h: None
        mod.get_axon_ntff_profile_hook = lambda: _hook
        import antenv

        antenv.axon_hooks = mod
        sys.modules["antenv.axon_hooks"] = mod
    except Exception:
        pass


LAST_EXEC_TIME_NS = None


def _prepare_in_maps(x, ve, sa_lambdas, qkv_w, c_proj_weight):
    x = np.asarray(x, dtype=np.float32)
    ve = np.asarray(ve, dtype=np.float32)
    sa_lambdas = np.asarray(sa_lambdas, dtype=np.float32)
    qkv_w = np.asarray(qkv_w, dtype=np.float32)
    c_proj_weight = np.asarray(c_proj_weight, dtype=np.float32)

    cc, ss = _rope_tables()
    mk = _masks()
    l0, l1 = float(sa_lambdas[0]), float(sa_lambdas[1])
    selm = np.zeros((4, 4 * P), dtype=np.float32)
    for w in range(4):
        selm[w, w * P:(w + 1) * P] = 1.0

    in_maps = []
    for c in range(8):
        b, g = c // 2, c % 2
        gs, ge = g * EG, (g + 1) * EG
        wq = qkv_w[0, gs:ge, :]           # [512, 1024]
        wk = qkv_w[1, gs:ge, :]
        wv = qkv_w[2, gs:ge, :] * l0      # fold lambda0 into the v projection
        in_maps.append({
            "xT": np.ascontiguousarray(x[b].T),                       # [D, T]
            "wqkT": np.ascontiguousarray(
                np.concatenate([wq, wk], axis=0).T),                  # [D, 1024]
            "wvT": np.ascontiguousarray(wv.T),                        # [D, 512]
            "ve": np.ascontiguousarray(
                ve[b].reshape(T, H, HD)[:, g * HG:(g + 1) * HG, :]
                .reshape(T, EG) * l1),                                # [T, 512]
            "cpT": np.ascontiguousarray(c_proj_weight[:, gs:ge].T),   # [512, D]
            "cc": cc, "ss": ss, "mk": mk, "selm": selm,
        })
    return in_maps


def kernel(x, ve, sa_lambdas, qkv_w, c_proj_weight):
    global LAST_EXEC_TIME_NS
    in_maps = _prepare_in_maps(x, ve, sa_lambdas, qkv_w, c_proj_weight)
    _try_install_profile_shim()
    nc = _get_nc()
    res = run_bass_kernel_spmd(nc, in_maps, core_ids=list(range(8)), trace=True)
    LAST_EXEC_TIME_NS = res.exec_time_ns

    outs = [res.results[c]["out"] for c in range(8)]
    full = np.stack([outs[2 * b] + outs[2 * b + 1] for b in range(B)], axis=0)
    return full.astype(np.float32)

